# revision 44
# baseline (speedup 1.0000x reference)
"""STFT (n_fft=4096, hop=1024, centered reflect-pad, windowed) on 8 TRN2 cores.

Algorithm: 2-stage Cooley-Tukey, n = 128*n1 + n2 (n1 in [0,32), n2 in [0,128)),
k = k1 + 32*k2 (k1 in [0,32), k2 in [0,64] for the 2049 kept bins).

Stage 1 packs BOTH complex planes and a 2-frame subgroup into the matmul
contraction: K = (plane, j, i, r) with n1 = 8j + i, r = frame parity.
lhsT = windowed frame data (stationary), rhs = a constant [128,128]
twiddle R12 -> psum [n2, (k1, comp, r)] in a single non-accumulating
matmul per subgroup (64 PE cycles/frame).

Host-side prep writes the frame data ALREADY in stage-1 lhsT layout as
fp16 ("frin"), so the input DMA is fully dense: 128 descriptors x 8KB
per 32-subgroup chunk (vs 512B gather packets when framing on-device).

Stage 2 runs q(=k1)-outer over ALL 516 frames at once with fp16 twiddle
blocks Gp/Gq reused across frame chunks; outputs accumulate in SBUF as
[128, 516] rows and DMA out with 2064B descriptors (f contiguous).

Partition map of stage-1 lhsT rows: p = 64*pl + 16*j + 2*i + r.
frin[p, 128*s + m] = xw_j[pl, 1024*(2s + r + j) + 128*i + m] where xw_j
is the j-th phase-windowed padded signal (built host-side, fp16).

Sharding: frame-parallel. Core i computes 516 frames starting at frame
512*i (SPMD, same NEFF); host trims/concatenates to 4097 global frames.
"""

import numpy as np

import concourse.bacc as bacc
import concourse.tile as tile
import concourse.mybir as mybir
from concourse import bass_utils

N_FFT = 4096
HOP = 1024
T = 4194304
NBINS = N_FFT // 2 + 1          # 2049
F_TOTAL = T // HOP + 1          # 4097
NCORES = 8

NF = 516                        # frames per core
NS = NF // 2                    # 258 subgroups of 2 frames
L = (NF - 1) * HOP + N_FFT      # per-core span of samples = 531456
SGROUPS1 = [8, 24, 32, 32, 32, 2]    # phase A1 input chunks (s 0..130)
SGROUPS2 = [32, 32, 32, 32]          # phase A2 input chunks (s 130..258)
FCHUNKS = ((0, 260), (260, 516))     # stage-2 frame chunks (A1/A2 split)

F32 = mybir.dt.float32
F16 = mybir.dt.float16

_cache = {}


def _host_constants():
    n1g = np.arange(32)
    k1g = np.arange(32)
    C = np.cos(2 * np.pi * np.outer(n1g, k1g) / 32)
    S = np.sin(2 * np.pi * np.outer(n1g, k1g) / 32)
    M = ((C, -S), (S, C))       # M[pl][comp]
    R12 = np.zeros((128, 128), np.float16)
    for pl in range(2):
        for c in range(2):
            blk = M[pl][c]      # [n1, k1]
            for j in range(4):
                for i in range(8):
                    for r in range(2):
                        p = 64 * pl + 16 * j + 2 * i + r
                        R12[p, 4 * k1g + 2 * c + r] = blk[8 * j + i]

    n2 = np.arange(128)
    k2 = np.arange(64)
    Gp = np.zeros((128, 32 * 128), np.float16)
    Gq = np.zeros((128, 32 * 128), np.float16)
    for q in range(32):
        kk = q + 32 * k2
        ang = 2 * np.pi * np.outer(n2, kk) / N_FFT
        gr = np.cos(ang)
        gi = -np.sin(ang)
        Gp[:, 128 * q:128 * q + 64] = gr.astype(np.float16)
        Gp[:, 128 * q + 64:128 * q + 128] = gi.astype(np.float16)
        Gq[:, 128 * q:128 * q + 64] = (-gi).astype(np.float16)
        Gq[:, 128 * q + 64:128 * q + 128] = gr.astype(np.float16)

    alt = ((-1.0) ** n2).astype(np.float16)
    E1 = np.zeros((128, 2), np.float16)
    E2 = np.zeros((128, 2), np.float16)
    E1[:, 0] = alt
    E2[:, 1] = alt
    return (R12, Gp, Gq, E1, E2)


def _build():
    nc = bacc.Bacc("TRN2", target_bir_lowering=False, debug=False,
                   enable_asserts=False, num_devices=NCORES)
    frin = nc.dram_tensor("frin", [128, 128 * NS], F16, kind="ExternalInput")
    r12 = nc.dram_tensor("r12", [128, 128], F16, kind="ExternalInput")
    gp = nc.dram_tensor("gp", [128, 32 * 128], F16, kind="ExternalInput")
    gq = nc.dram_tensor("gq", [128, 32 * 128], F16, kind="ExternalInput")
    e1 = nc.dram_tensor("e1", [128, 2], F16, kind="ExternalInput")
    e2 = nc.dram_tensor("e2", [128, 2], F16, kind="ExternalInput")
    # outputs in fp16 (host upcasts): halves the output DMA traffic, and the
    # fp16 quantization (~2e-4 rel) is far inside the accuracy budget
    out = nc.dram_tensor("o", [2, 2048, NF], F16, kind="ExternalOutput")
    oute = nc.dram_tensor("oe", [2, 1, NF], F16, kind="ExternalOutput")

    with tile.TileContext(nc) as tc:
        with (
            tc.tile_pool(name="const", bufs=1) as cpool,
            tc.tile_pool(name="fr", bufs=7) as frpool,
            tc.tile_pool(name="ys", bufs=1) as yspool,
            tc.tile_pool(name="ost", bufs=17) as ostpool,
            tc.tile_pool(name="ps1", bufs=3, space="PSUM") as ps1pool,
            tc.tile_pool(name="ps2", bufs=4, space="PSUM") as ps2pool,
            tc.tile_pool(name="pse", bufs=1, space="PSUM") as psepool,
        ):
            t_r12 = cpool.tile([128, 128], F16, tag="r12")
            t_gp = cpool.tile([128, 32 * 128], F16, tag="gp")
            t_gq = cpool.tile([128, 32 * 128], F16, tag="gq")
            t_e1 = cpool.tile([128, 2], F16, tag="e1")
            t_e2 = cpool.tile([128, 2], F16, tag="e2")
            # r12 is needed by the first matmul — load it first on the sync
            # queue ahead of the frin chunks. The stage-2 constants load on
            # the scalar queue once the first frin chunks are in flight, so
            # they neither starve the pipeline-critical first chunk nor
            # queue ahead of phase-B output writes.
            nc.sync.dma_start(t_r12[:], r12.ap()[:, :])

            t_ys = yspool.tile([128, 128 * NS], F16, tag="ys")
            # ys layout: col = (2*k1 + c)*NF + f — frame-major per (k1,c) so
            # stage-2 matmul rhs reads are contiguous. The phase-A copies
            # scatter psum cols (s, k1, c, r) into it.
            ysw = t_ys[:, :].rearrange("p (k c s2 r) -> p s2 k c r",
                                       k=32, c=2, s2=NS, r=2)

            cp_ix = 0

            def emit_group(fr, s0, ns):
                nonlocal cp_ix
                for b in range(0, ns, 4):
                    nb = min(4, ns - b)
                    ps = ps1pool.tile([128, 512], F32, tag="ps1")
                    for t in range(nb):
                        nc.tensor.matmul(ps[:, 128 * t:128 * t + 128],
                                         fr[:, 128 * (b + t):128 * (b + t + 1)],
                                         t_r12[:], start=True, stop=True)
                    # scatter on the read side: the fp16 writes stay
                    # contiguous per (k1,c) run so the cast can stream
                    dst = ysw[:, s0 + b:s0 + b + nb].rearrange(
                        "p s k c r -> p k c s r")
                    srcv = ps[:, 0:128 * nb].rearrange(
                        "p (s k c r) -> p k c s r", s=nb, k=32, c=2, r=2)
                    if cp_ix % 2 == 0:
                        nc.vector.tensor_copy(dst, srcv)
                    else:
                        nc.scalar.copy(dst, srcv)
                    cp_ix += 1

            def emit_s2_qp(ci, qp, osts):
                fa, fb = FCHUNKS[ci]
                n = fb - fa
                for t in range(2):
                    q = 2 * qp + t
                    ps = ps2pool.tile([128, 260], F32, tag="ps2")
                    nc.tensor.matmul(ps[:, 0:n],
                                     t_gp[:, 128 * q:128 * q + 128],
                                     t_ys[:, 2 * q * NF + fa:
                                          2 * q * NF + fb],
                                     start=True, stop=False)
                    nc.tensor.matmul(ps[:, 0:n],
                                     t_gq[:, 128 * q:128 * q + 128],
                                     t_ys[:, (2 * q + 1) * NF + fa:
                                          (2 * q + 1) * NF + fb],
                                     start=False, stop=True)
                    dst = osts[qp][:, NF * t + fa:NF * t + fb]
                    if (qp + t) % 2 == 0:
                        nc.vector.tensor_copy(dst, ps[:, 0:n])
                    else:
                        nc.scalar.copy(dst, ps[:, 0:n])
                # fire this q-pair's slice now — both chunk widths keep the
                # descriptors >= 512B, and chunk-0 writes fill the DMA hole
                # between input-end and phase B2
                dstq = out.ap().rearrange(
                    "c (k q) b -> (c k) q b",
                    q=32)[:, 2 * qp:2 * qp + 2, fa:fb]
                srcq = osts[qp][:].rearrange(
                    "p (q b) -> p q b", b=NF)[:, :, fa:fb]
                if qp % 2 == 0:
                    nc.sync.dma_start(dstq, srcq)
                else:
                    nc.gpsimd.dma_start(dstq, srcq)

            def emit_bin2048(ci, oste):
                # bin 2048 (k1=0, k2=64): +/- sum over n2 of Y[0]
                fa, fb = FCHUNKS[ci]
                n = fb - fa
                pse = psepool.tile([2, 260], F32, tag="pse")
                nc.tensor.matmul(pse[:, 0:n], t_e1[:], t_ys[:, fa:fb],
                                 start=True, stop=False)
                nc.tensor.matmul(pse[:, 0:n], t_e2[:],
                                 t_ys[:, NF + fa:NF + fb],
                                 start=False, stop=True)
                nc.vector.tensor_copy(oste[:, fa:fb], pse[:, 0:n])
                nc.sync.dma_start(oute.ap()[:, 0, fa:fb], oste[:, fa:fb])

            # Warm-up: keep the PE busy while the first frin chunks stream
            # in, so the clock is fully ramped when real work starts. The
            # results are never read.
            warm = ps1pool.tile([128, 512], F32, tag="ps1")
            for w in range(48):
                nc.tensor.matmul(warm[:, 0:128], t_r12[:], t_r12[:],
                                 start=True, stop=True)

            # ---- Phase A1: first half of the frames (s 0..128) ----
            s0 = 0
            for g, ns in enumerate(SGROUPS1):
                fr = frpool.tile([128, 128 * 32], F16, tag="fr")
                nc.sync.dma_start(fr[:, 0:128 * ns],
                                  frin.ap()[:, 128 * s0:128 * (s0 + ns)])
                if g == 4:
                    nc.scalar.dma_start(t_gp[:], gp.ap()[:, :])
                    nc.scalar.dma_start(t_gq[:], gq.ap()[:, :])
                    nc.scalar.dma_start(t_e1[:], e1.ap()[:, :])
                    nc.scalar.dma_start(t_e2[:], e2.ap()[:, :])
                emit_group(fr[:, 0:128 * ns], s0, ns)
                s0 += ns

            # queue all phase-A2 input DMAs now: the transfers stream in
            # while the PE/copy engines run stage 2 on the first half
            frs2 = []
            for ns in SGROUPS2:
                fr = frpool.tile([128, 128 * 32], F16, tag="fr")
                nc.sync.dma_start(fr[:, 0:128 * ns],
                                  frin.ap()[:, 128 * s0:128 * (s0 + ns)])
                frs2.append((fr, s0, ns))
                s0 += ns

            # ---- Phase B1: stage 2 over frames [0, 260) ----
            osts = [ostpool.tile([128, 2 * NF], F16, tag="ost",
                                 name=f"ost{i}") for i in range(16)]
            oste = ostpool.tile([2, NF], F16, tag="oste")
            for qp in range(16):
                emit_s2_qp(0, qp, osts)
            emit_bin2048(0, oste)

            # ---- Phase A2: second half of the frames (s 130..258) ----
            for fr, s0g, ns in frs2:
                emit_group(fr[:, 0:128 * ns], s0g, ns)

            # ---- Phase B2: stage 2 over frames [260, 516) + output DMA ----
            for qp in range(16):
                emit_s2_qp(1, qp, osts)
            emit_bin2048(1, oste)

    nc.compile()
    return nc


def _prep_inputs(x, window):
    pad = N_FFT // 2
    xp = np.pad(np.asarray(x), ((0, 0), (pad, pad)), mode="reflect")
    total = xp.shape[1]
    need = (NCORES - 1) * 512 * HOP + L + 8192
    xp_ext = np.zeros((2, max(total, need)), np.float32)
    xp_ext[:, :total] = xp
    w = np.asarray(window, np.float32)
    reps = xp_ext.shape[1] // HOP + 1
    xws = []
    for j in range(4):
        wj = np.tile(w[HOP * j:HOP * (j + 1)], reps)[:xp_ext.shape[1]]
        xws.append(xp_ext * wj[None, :])

    frins = []
    for i in range(NCORES):
        s0 = i * 512 * HOP
        fr = np.empty((128, 128 * NS), np.float16)
        for pl in range(2):
            for j in range(4):
                seg = xws[j][pl]
                v = np.lib.stride_tricks.as_strided(
                    seg[s0 + 1024 * j:], (8, 2, NS, 128),
                    (128 * 4, 1024 * 4, 2048 * 4, 4))
                fr[64 * pl + 16 * j:64 * pl + 16 * j + 16] = \
                    v.reshape(16, 128 * NS)
        frins.append(fr)
    return frins


def kernel(x, window):
    import time
    t0 = time.time()
    x = np.asarray(x, np.float32)
    window = np.asarray(window, np.float32)
    if "nc" not in _cache:
        _cache["nc"] = _build()
    nc = _cache["nc"]
    print(f"[kernel] build done {time.time()-t0:.2f}s", flush=True)

    frins = _prep_inputs(x, window)
    R12, Gp, Gq, E1, E2 = _host_constants()

    in_maps = []
    for i in range(NCORES):
        in_maps.append({"frin": frins[i], "r12": R12, "gp": Gp, "gq": Gq,
                        "e1": E1, "e2": E2})

    print(f"[kernel] inputs prepped {time.time()-t0:.2f}s", flush=True)
    import os
    trace = bool(os.environ.get("KERNEL_TRACE"))
    res = bass_utils.run_bass_kernel_spmd(nc, in_maps,
                                          core_ids=list(range(NCORES)),
                                          trace=trace)
    if trace and res.exec_time_ns is not None:
        global LAST_EXEC_NS
        LAST_EXEC_NS = res.exec_time_ns
        print(f"[kernel] exec_time_ns={res.exec_time_ns}", flush=True)
        if res.instructions_and_trace is not None:
            print(f"[kernel] trace: {res.instructions_and_trace[1]}",
                  flush=True)
    print(f"[kernel] spmd done {time.time()-t0:.2f}s", flush=True)
    out = np.zeros((2, NBINS, F_TOTAL), np.float32)
    for i in range(NCORES):
        o = res.results[i]["o"]
        oe = res.results[i]["oe"]
        f0 = 512 * i
        nf = 513 if i == NCORES - 1 else 512
        out[:, :2048, f0:f0 + nf] = o[:, :, :nf].astype(np.float32)
        out[:, 2048, f0:f0 + nf] = oe[:, 0, :nf].astype(np.float32)
    return out


# revision 45
# speedup vs baseline: 1.0442x; 1.0442x over previous
"""STFT (n_fft=4096, hop=1024, centered reflect-pad, windowed) on 8 TRN2 cores.

Algorithm: 2-stage Cooley-Tukey, n = 128*n1 + n2 (n1 in [0,32), n2 in [0,128)),
k = k1 + 32*k2 (k1 in [0,32), k2 in [0,64] for the 2049 kept bins).

Stage 1 packs BOTH complex planes and a 2-frame subgroup into the matmul
contraction: K = (plane, j, i, r) with n1 = 8j + i, r = frame parity.
lhsT = windowed frame data (stationary), rhs = a constant [128,128]
twiddle R12 -> psum [n2, (k1, comp, r)] in a single non-accumulating
matmul per subgroup (64 PE cycles/frame).

Host-side prep writes the frame data ALREADY in stage-1 lhsT layout as
fp16 ("frin"), so the input DMA is fully dense: 128 descriptors x 8KB
per 32-subgroup chunk (vs 512B gather packets when framing on-device).

Stage 1 output lands in SBUF ("ys") frame-major per (k1, comp) so the
stage-2 matmul rhs reads are fully contiguous; the psum->ys casts do the
scatter on their read side.

Stage 2 runs q(=k1)-outer with fp16 twiddle blocks Gp/Gq, split into two
frame chunks pipelined against the two input halves: A1 (s 0..130) ->
B1 (frames 0..260, its output slices stream out during A2's input DMA)
-> A2 (s 130..258) -> B2 (frames 260..516). Outputs are written fp16
(host upcasts) with >=512B descriptors; bin 2048 is a separate +/-1
matmul. A dummy warm-up matmul stream ramps the PE clock while the
first input chunk lands.

Partition map of stage-1 lhsT rows: p = 64*pl + 16*j + 2*i + r.
frin[p, 128*s + m] = xw_j[pl, 1024*(2s + r + j) + 128*i + m] where xw_j
is the j-th phase-windowed padded signal (built host-side, fp16).

Sharding: frame-parallel. Core i computes 516 frames starting at frame
512*i (SPMD, same NEFF); host trims/concatenates to 4097 global frames.
"""

import numpy as np

import concourse.bacc as bacc
import concourse.tile as tile
import concourse.mybir as mybir
from concourse import bass_utils

N_FFT = 4096
HOP = 1024
T = 4194304
NBINS = N_FFT // 2 + 1          # 2049
F_TOTAL = T // HOP + 1          # 4097
NCORES = 8

NF = 516                        # frames per core
NS = NF // 2                    # 258 subgroups of 2 frames
L = (NF - 1) * HOP + N_FFT      # per-core span of samples = 531456
SGROUPS1 = [8, 24, 32, 32, 32, 2]    # phase A1 input chunks (s 0..130)
SGROUPS2 = [32, 32, 32, 32]          # phase A2 input chunks (s 130..258)
FCHUNKS = ((0, 260), (260, 516))     # stage-2 frame chunks (A1/A2 split)

F32 = mybir.dt.float32
F16 = mybir.dt.float16

_cache = {}


def _host_constants():
    n1g = np.arange(32)
    k1g = np.arange(32)
    C = np.cos(2 * np.pi * np.outer(n1g, k1g) / 32)
    S = np.sin(2 * np.pi * np.outer(n1g, k1g) / 32)
    M = ((C, -S), (S, C))       # M[pl][comp]
    R12 = np.zeros((128, 128), np.float16)
    for pl in range(2):
        for c in range(2):
            blk = M[pl][c]      # [n1, k1]
            for j in range(4):
                for i in range(8):
                    for r in range(2):
                        p = 64 * pl + 16 * j + 2 * i + r
                        R12[p, 4 * k1g + 2 * c + r] = blk[8 * j + i]

    n2 = np.arange(128)
    k2 = np.arange(64)
    Gp = np.zeros((128, 32 * 128), np.float16)
    Gq = np.zeros((128, 32 * 128), np.float16)
    for q in range(32):
        kk = q + 32 * k2
        ang = 2 * np.pi * np.outer(n2, kk) / N_FFT
        gr = np.cos(ang)
        gi = -np.sin(ang)
        Gp[:, 128 * q:128 * q + 64] = gr.astype(np.float16)
        Gp[:, 128 * q + 64:128 * q + 128] = gi.astype(np.float16)
        Gq[:, 128 * q:128 * q + 64] = (-gi).astype(np.float16)
        Gq[:, 128 * q + 64:128 * q + 128] = gr.astype(np.float16)

    alt = ((-1.0) ** n2).astype(np.float16)
    E1 = np.zeros((128, 2), np.float16)
    E2 = np.zeros((128, 2), np.float16)
    E1[:, 0] = alt
    E2[:, 1] = alt
    return (R12, Gp, Gq, E1, E2)


def _build():
    nc = bacc.Bacc("TRN2", target_bir_lowering=False, debug=False,
                   enable_asserts=False, num_devices=NCORES)
    frin = nc.dram_tensor("frin", [128, 128 * NS], F16, kind="ExternalInput")
    r12 = nc.dram_tensor("r12", [128, 128], F16, kind="ExternalInput")
    gp = nc.dram_tensor("gp", [128, 32 * 128], F16, kind="ExternalInput")
    gq = nc.dram_tensor("gq", [128, 32 * 128], F16, kind="ExternalInput")
    e1 = nc.dram_tensor("e1", [128, 2], F16, kind="ExternalInput")
    e2 = nc.dram_tensor("e2", [128, 2], F16, kind="ExternalInput")
    # outputs in fp16 (host upcasts): halves the output DMA traffic, and the
    # fp16 quantization (~2e-4 rel) is far inside the accuracy budget
    out = nc.dram_tensor("o", [2, 2048, NF], F16, kind="ExternalOutput")
    oute = nc.dram_tensor("oe", [2, 1, NF], F16, kind="ExternalOutput")

    with tile.TileContext(nc) as tc:
        with (
            tc.tile_pool(name="const", bufs=1) as cpool,
            tc.tile_pool(name="fr", bufs=7) as frpool,
            tc.tile_pool(name="ys", bufs=1) as yspool,
            tc.tile_pool(name="ost", bufs=17) as ostpool,
            tc.tile_pool(name="ps1", bufs=3, space="PSUM") as ps1pool,
            tc.tile_pool(name="ps2", bufs=4, space="PSUM") as ps2pool,
            tc.tile_pool(name="pse", bufs=1, space="PSUM") as psepool,
        ):
            t_r12 = cpool.tile([128, 128], F16, tag="r12")
            t_gp = cpool.tile([128, 32 * 128], F16, tag="gp")
            t_gq = cpool.tile([128, 32 * 128], F16, tag="gq")
            t_e1 = cpool.tile([128, 2], F16, tag="e1")
            t_e2 = cpool.tile([128, 2], F16, tag="e2")
            # r12 is needed by the first matmul — load it first on the sync
            # queue ahead of the frin chunks. The stage-2 constants load on
            # the scalar queue once the first frin chunks are in flight, so
            # they neither starve the pipeline-critical first chunk nor
            # queue ahead of phase-B output writes.
            nc.sync.dma_start(t_r12[:], r12.ap()[:, :])

            t_ys = yspool.tile([128, 128 * NS], F16, tag="ys")
            # ys layout: col = (2*k1 + c)*NF + f — frame-major per (k1,c) so
            # stage-2 matmul rhs reads are contiguous. The phase-A copies
            # scatter psum cols (s, k1, c, r) into it.
            ysw = t_ys[:, :].rearrange("p (k c s2 r) -> p s2 k c r",
                                       k=32, c=2, s2=NS, r=2)

            cp_ix = 0

            def emit_group(fr, s0, ns):
                nonlocal cp_ix
                for b in range(0, ns, 4):
                    nb = min(4, ns - b)
                    ps = ps1pool.tile([128, 512], F32, tag="ps1")
                    for t in range(nb):
                        nc.tensor.matmul(ps[:, 128 * t:128 * t + 128],
                                         fr[:, 128 * (b + t):128 * (b + t + 1)],
                                         t_r12[:], start=True, stop=True)
                    # scatter on the read side: the fp16 writes stay
                    # contiguous per (k1,c) run so the cast can stream
                    dst = ysw[:, s0 + b:s0 + b + nb].rearrange(
                        "p s k c r -> p k c s r")
                    srcv = ps[:, 0:128 * nb].rearrange(
                        "p (s k c r) -> p k c s r", s=nb, k=32, c=2, r=2)
                    if cp_ix % 2 == 0:
                        nc.vector.tensor_copy(dst, srcv)
                    else:
                        nc.scalar.copy(dst, srcv)
                    cp_ix += 1

            def emit_s2_qp(ci, qp, osts):
                fa, fb = FCHUNKS[ci]
                n = fb - fa
                for t in range(2):
                    q = 2 * qp + t
                    ps = ps2pool.tile([128, 260], F32, tag="ps2")
                    nc.tensor.matmul(ps[:, 0:n],
                                     t_gp[:, 128 * q:128 * q + 128],
                                     t_ys[:, 2 * q * NF + fa:
                                          2 * q * NF + fb],
                                     start=True, stop=False)
                    nc.tensor.matmul(ps[:, 0:n],
                                     t_gq[:, 128 * q:128 * q + 128],
                                     t_ys[:, (2 * q + 1) * NF + fa:
                                          (2 * q + 1) * NF + fb],
                                     start=False, stop=True)
                    dst = osts[qp][:, NF * t + fa:NF * t + fb]
                    if (qp + t) % 2 == 0:
                        nc.vector.tensor_copy(dst, ps[:, 0:n])
                    else:
                        nc.scalar.copy(dst, ps[:, 0:n])
                # fire this q-pair's slice now — both chunk widths keep the
                # descriptors >= 512B, and chunk-0 writes fill the DMA hole
                # between input-end and phase B2
                dstq = out.ap().rearrange(
                    "c (k q) b -> (c k) q b",
                    q=32)[:, 2 * qp:2 * qp + 2, fa:fb]
                srcq = osts[qp][:].rearrange(
                    "p (q b) -> p q b", b=NF)[:, :, fa:fb]
                if qp % 2 == 0:
                    nc.sync.dma_start(dstq, srcq)
                else:
                    nc.gpsimd.dma_start(dstq, srcq)

            def emit_bin2048(ci, oste):
                # bin 2048 (k1=0, k2=64): +/- sum over n2 of Y[0]
                fa, fb = FCHUNKS[ci]
                n = fb - fa
                pse = psepool.tile([2, 260], F32, tag="pse")
                nc.tensor.matmul(pse[:, 0:n], t_e1[:], t_ys[:, fa:fb],
                                 start=True, stop=False)
                nc.tensor.matmul(pse[:, 0:n], t_e2[:],
                                 t_ys[:, NF + fa:NF + fb],
                                 start=False, stop=True)
                nc.vector.tensor_copy(oste[:, fa:fb], pse[:, 0:n])
                nc.sync.dma_start(oute.ap()[:, 0, fa:fb], oste[:, fa:fb])

            # Warm-up: keep the PE busy while the first frin chunks stream
            # in, so the clock is fully ramped when real work starts. The
            # results are never read.
            warm = ps1pool.tile([128, 512], F32, tag="ps1")
            for w in range(48):
                nc.tensor.matmul(warm[:, 0:128], t_r12[:], t_r12[:],
                                 start=True, stop=True)

            # ---- Phase A1: first half of the frames (s 0..128) ----
            s0 = 0
            for g, ns in enumerate(SGROUPS1):
                fr = frpool.tile([128, 128 * 32], F16, tag="fr")
                nc.sync.dma_start(fr[:, 0:128 * ns],
                                  frin.ap()[:, 128 * s0:128 * (s0 + ns)])
                if g == 4:
                    nc.scalar.dma_start(t_gp[:], gp.ap()[:, :])
                    nc.scalar.dma_start(t_gq[:], gq.ap()[:, :])
                    nc.scalar.dma_start(t_e1[:], e1.ap()[:, :])
                    nc.scalar.dma_start(t_e2[:], e2.ap()[:, :])
                emit_group(fr[:, 0:128 * ns], s0, ns)
                s0 += ns

            # queue all phase-A2 input DMAs now: the transfers stream in
            # while the PE/copy engines run stage 2 on the first half
            frs2 = []
            for ns in SGROUPS2:
                fr = frpool.tile([128, 128 * 32], F16, tag="fr")
                nc.sync.dma_start(fr[:, 0:128 * ns],
                                  frin.ap()[:, 128 * s0:128 * (s0 + ns)])
                frs2.append((fr, s0, ns))
                s0 += ns

            # ---- Phase B1: stage 2 over frames [0, 260) ----
            osts = [ostpool.tile([128, 2 * NF], F16, tag="ost",
                                 name=f"ost{i}") for i in range(16)]
            oste = ostpool.tile([2, NF], F16, tag="oste")
            for qp in range(16):
                emit_s2_qp(0, qp, osts)
            emit_bin2048(0, oste)

            # ---- Phase A2: second half of the frames (s 130..258) ----
            for fr, s0g, ns in frs2:
                emit_group(fr[:, 0:128 * ns], s0g, ns)

            # ---- Phase B2: stage 2 over frames [260, 516) + output DMA ----
            for qp in range(16):
                emit_s2_qp(1, qp, osts)
            emit_bin2048(1, oste)

    nc.compile()
    return nc


def _prep_inputs(x, window):
    pad = N_FFT // 2
    xp = np.pad(np.asarray(x), ((0, 0), (pad, pad)), mode="reflect")
    total = xp.shape[1]
    need = (NCORES - 1) * 512 * HOP + L + 8192
    xp_ext = np.zeros((2, max(total, need)), np.float32)
    xp_ext[:, :total] = xp
    w = np.asarray(window, np.float32)
    reps = xp_ext.shape[1] // HOP + 1
    xws = []
    for j in range(4):
        wj = np.tile(w[HOP * j:HOP * (j + 1)], reps)[:xp_ext.shape[1]]
        xws.append(xp_ext * wj[None, :])

    frins = []
    for i in range(NCORES):
        s0 = i * 512 * HOP
        fr = np.empty((128, 128 * NS), np.float16)
        for pl in range(2):
            for j in range(4):
                seg = xws[j][pl]
                v = np.lib.stride_tricks.as_strided(
                    seg[s0 + 1024 * j:], (8, 2, NS, 128),
                    (128 * 4, 1024 * 4, 2048 * 4, 4))
                fr[64 * pl + 16 * j:64 * pl + 16 * j + 16] = \
                    v.reshape(16, 128 * NS)
        frins.append(fr)
    return frins


def kernel(x, window):
    import time
    t0 = time.time()
    x = np.asarray(x, np.float32)
    window = np.asarray(window, np.float32)
    if "nc" not in _cache:
        _cache["nc"] = _build()
    nc = _cache["nc"]
    print(f"[kernel] build done {time.time()-t0:.2f}s", flush=True)

    frins = _prep_inputs(x, window)
    R12, Gp, Gq, E1, E2 = _host_constants()

    in_maps = []
    for i in range(NCORES):
        in_maps.append({"frin": frins[i], "r12": R12, "gp": Gp, "gq": Gq,
                        "e1": E1, "e2": E2})

    print(f"[kernel] inputs prepped {time.time()-t0:.2f}s", flush=True)
    import os
    trace = bool(os.environ.get("KERNEL_TRACE"))
    res = bass_utils.run_bass_kernel_spmd(nc, in_maps,
                                          core_ids=list(range(NCORES)),
                                          trace=trace)
    if trace and res.exec_time_ns is not None:
        global LAST_EXEC_NS
        LAST_EXEC_NS = res.exec_time_ns
        print(f"[kernel] exec_time_ns={res.exec_time_ns}", flush=True)
        if res.instructions_and_trace is not None:
            print(f"[kernel] trace: {res.instructions_and_trace[1]}",
                  flush=True)
    print(f"[kernel] spmd done {time.time()-t0:.2f}s", flush=True)
    out = np.zeros((2, NBINS, F_TOTAL), np.float32)
    for i in range(NCORES):
        o = res.results[i]["o"]
        oe = res.results[i]["oe"]
        f0 = 512 * i
        nf = 513 if i == NCORES - 1 else 512
        out[:, :2048, f0:f0 + nf] = o[:, :, :nf].astype(np.float32)
        out[:, 2048, f0:f0 + nf] = oe[:, 0, :nf].astype(np.float32)
    return out
